# revision 42
# baseline (speedup 1.0000x reference)
"""Trainium2 Bass kernel for nn_Encoder (GRU + input attention).

Shapes (hardcoded): B=32, T=128, N=256, H=512; 8 NeuronCores, batch
sharded 4 examples/core.

Math (matching the reference):
  hs = GRU scan over T steps (Keras GRUCell, reset_after=True, gates z,r,h)
  s[b,n,u]  = sum_t data[b,t,n] w1_w[t,u] + w1_b[u]
  h[t,b,u]  = hs[t,b,:] @ w2_w + w2_b[u]
  score[t,b,n] = sum_u v[u] tanh(s + h)   (+v_b: softmax-invariant)
  alpha = softmax_n(score);  out[b,t,:] = data[b,t,:] * alpha[...]

Key restructure vs the 317us baseline: tanh(s+h) is replaced by a
separable shifted-tanh model fitted offline (fixed universal constants):
  tanh(s+h) ~= c0*tanh(a0*s+b0) + d0
             + sum_k [tanh(s+A_k)-tanh(s+A_{k+1})] * (c_k*tanh(kap_k*h+mu_k) + d_k)
so score becomes K+1=14 PE matmuls per example (stationary = tanh
features of h laid [u,t], moving = v-weighted s-features [u,n]) instead
of a 134M-element e=tanh tensor. Removes ~1.2us/step of Activation work
and all per-step e-adds/score matmuls from the scan steady state.
End-to-end error (incl f16 + fp8 GRU weights): ~6.7e-3 rel.

GRU scan structure per step (latency-optimized):
 - h(t+1)@R is evaluated as two matmul waves (z(.)h)@R + ((1-z)(.)c)@R so
   wave A runs during the r/c gate chain and only wave B (16 r-bank
   matmuls) gates sigmoid_r of the next step; the h-assembly is off the
   critical path.
 - All gate nonlinearities are Tanh (sigma(x)=(1+tanh(x/2))/2 folded via
   host-side scaling: K/input-biases doubled, R_h halved, ACT input
   scales 0.25/0.5) so one activation table set (exp_and_others:
   Tanh+Exp) serves the whole kernel - zero table swaps.
 - wave movings mA=(zt'+1)(.)h=2zh and mB=(zt'-1)(.)(-c)=2(1-z)c are one
   scalar_tensor_tensor op each; cgN=-c comes free via ACT scale=-0.5.
Attention background (s-features, D tensors) threads into scan idle gaps
at 1 op per 2 steps; hp blocks stay lagged 1 block behind the scan; the
W_k=tanh(kap_k*hp+..) features + 14 score matmuls + softmax + final
multiply run in a pipelined epilogue. Small constants ride in two
batched DMA blobs to cut HWDGE serialization in the prologue.
"""

import os
import sys

import numpy as np

# concourse (Bass) lives in the TRN2 container; make sure it's importable
for _p in ("/root/.axon_site", "/root/.axon_site/_ro/trn_rl_repo",
           "/root/.axon_site/_ro/pypackages", "/opt/trn_rl_repo",
           "/opt/pypackages"):
    if os.path.isdir(_p) and _p not in sys.path:
        sys.path.append(_p)

B, T, N, H = 32, 128, 256, 512
NC = 8           # cores
BL = B // NC     # batch per core (4)
H3 = 3 * H

_CACHE = {}
_CHAIN_FIX = []
DEBUG = False
SCAN_ONLY = os.environ.get("NN_ENC_SCAN_ONLY", "0") == "1"

# ---- separable tanh(s+h) model constants (fitted offline; universal) ----
SEP_K = 13
SEP_A = [-3.2709594, -2.5955656, -1.9867907, -1.3730230, -0.8524583,
         -0.4658397, -0.1405748, 0.1436354, 0.4704521, 0.8618943,
         1.3933731, 2.0088742, 2.6093273, 3.3204157]
SEP_C = [-0.5496367, -0.5309651, -0.5037420, -0.5256692, -0.4968104,
         -0.5381408, -0.5060670, -0.5380091, -0.4986397, -0.5247371,
         -0.5046583, -0.5389031, -0.5067456]
SEP_D = [0.5366726, 0.5316761, 0.5094128, 0.5085504, 0.4964807,
         0.4820233, 0.5037248, 0.5113558, 0.5045732, 0.4939719,
         0.4838700, 0.4722759, 0.5153870]
SEP_KAP = [3.1443172, 3.3217070, 3.4944437, 3.7533102, 4.4334431,
           4.6020584, 5.0746264, 4.5910144, 4.3981624, 3.7172728,
           3.4599097, 3.2798653, 3.1275754]
SEP_MU = [9.3775816, 7.7132607, 5.9568019, 4.2126617, 2.9646783,
          1.4329745, -0.0097532, -1.4501734, -2.9762077, -4.2356744,
          -5.9752331, -7.6982388, -9.3694906]
SEP_C0 = 0.9994611
SEP_A0 = 1.0115169
SEP_B0 = 3.3683648

# smallf16 blob column offsets
O_W1, O_W2, O_ID, O_ONES, O_BRECH, O_H02, O_H016 = 0, 128, 640, 768, 896, 912, 928
S16_COLS = 944
# smallf32 blob column offsets: bzr 0:8, bh 8:12, w1b 12, bW 13:26, bA 26:41, v 41
S32_COLS = 42


def _build():
    import concourse.bass as bass
    import concourse.bacc as bacc
    import concourse.tile as tile
    import concourse.mybir as mybir

    f16 = mybir.dt.float16
    f32 = mybir.dt.float32
    Alu = mybir.AluOpType
    Act = mybir.ActivationFunctionType

    nc = bacc.Bacc("TRN2", target_bir_lowering=False, debug=False)

    # ---- dram I/O ----
    d_data16 = nc.dram_tensor("data16", [T, BL, N], f16, kind="ExternalInput")
    d_dataout = nc.dram_tensor("dataout", [T, BL, N], f32, kind="ExternalInput")
    f8 = mybir.dt.float8e4
    d_R8 = nc.dram_tensor("R8_l", [128, 4, 8, 128], f8, kind="ExternalInput")
    d_Rh = nc.dram_tensor("Rh_l", [128, 4, 4, 128], f16, kind="ExternalInput")
    d_K = nc.dram_tensor("K_l", [128, 12, 2, 128], f16, kind="ExternalInput")
    d_s16 = nc.dram_tensor("small16", [128, S16_COLS], f16, kind="ExternalInput")
    d_s32 = nc.dram_tensor("small32", [128, S32_COLS], f32, kind="ExternalInput")
    d_out = nc.dram_tensor("out", [BL, T, N], f32, kind="ExternalOutput")
    if DEBUG:
        d_hs = nc.dram_tensor("hs_dump", [128, T + 1, 16], f16,
                              kind="ExternalOutput")
        d_sxd = nc.dram_tensor("sx_dump", [128, BL, N], f16,
                               kind="ExternalOutput")
        d_hpd = nc.dram_tensor("hp_dump", [128, 32, 4, 4], f16,
                               kind="ExternalOutput")
        d_d0d = nc.dram_tensor("d0_dump", [128, BL, N], f16,
                               kind="ExternalOutput")

    with tile.TileContext(nc) as tc:
        with (
            tc.tile_pool(name="const", bufs=1) as cpool,
            tc.tile_pool(name="work", bufs=4) as wpool,
            tc.tile_pool(name="tsbuf", bufs=3) as tspool,
            tc.tile_pool(name="gater", bufs=2, space="PSUM") as grpool,
            tc.tile_pool(name="gatezh", bufs=2, space="PSUM") as gzpool,
            tc.tile_pool(name="bigps", bufs=2, space="PSUM") as bpool,
            tc.tile_pool(name="score", bufs=1, space="PSUM") as scpool,
        ):
            # ---- persistent tiles ----
            t_R8 = cpool.tile([128, 4, 8, 128], f8)
            t_Rh = cpool.tile([128, 4, 4, 128], f16)
            t_K = cpool.tile([128, 12, 2, 128], f16)
            t_s16 = cpool.tile([128, S16_COLS], f16)
            t_s32 = cpool.tile([128, S32_COLS], f32)
            t_d16 = cpool.tile([128, BL, N], f16)          # data [t, b, n]
            t_dout = cpool.tile([128, BL, N], f32)         # dataout [tt, b, n]
            t_dT = cpool.tile([128, 2, BL, 128], f16)      # dataT [p, nc, b, t]
            t_addmx = cpool.tile([128, 32, T], f16)        # 2*mx_z' | 2*mx_r'
            t_xh = cpool.tile([128, 16, T], f16)           # 2*xh' per t
            t_sxs = cpool.tile([128, BL, N], f16)          # s = sx + w1_b
            t_hs = cpool.tile([128, T + 1, 16], f16)       # h^T packed
            t_hp = cpool.tile([128, 32, 4, 4], f16)        # hp [u, blk, tl, b]
            t_W = [cpool.tile([128, 32, 4, 4], f16, tag=f"W_{k}",
                              name=f"W_{k}") for k in range(SEP_K)]
            # epilogue stationary pad: [zeros(4 blks) | W blocks 28:32] so a
            # 32-wide stationary at tile_position row 96 adds only rows
            # 112:128 (PE tile_position rows are restricted to multiples
            # of 32).
            t_We = [cpool.tile([128, 8, 4, 4], f16, tag=f"We_{k}",
                               name=f"We_{k}") for k in range(SEP_K)]
            t_D = [cpool.tile([128, BL, N], f16, tag=f"D_{k}",
                              name=f"D_{k}") for k in range(SEP_K)]
            t_X = cpool.tile([128, BL, N], f16)            # D0 accumulator
            t_ssum = cpool.tile([128, BL], f32)
            t_rinv = cpool.tile([128, BL], f32)

            W1 = t_s16[:, O_W1:O_W1 + 128]
            IDENT = t_s16[:, O_ID:O_ID + 128]
            ONES = t_s16[:, O_ONES:O_ONES + 128]
            BRECH = t_s16[:, O_BRECH:O_BRECH + 16]
            H02 = t_s16[:, O_H02:O_H02 + 16]

            def W2(hc):
                return t_s16[:, O_W2 + 128 * hc:O_W2 + 128 * (hc + 1)]

            V_AP = t_s32[:, 41:42]

            # ---- DMA in. Order sets transfer priority: the scan start is
            # gated by d16 (-> transposes -> mx) and R8's r-chunks (first
            # wave's r bank); Rh/K2 only matter a few hundred ns later.
            nc.sync.dma_start(out=t_s16[:, :], in_=d_s16.ap()[:, :])
            nc.sync.dma_start(out=t_s32[:, :], in_=d_s32.ap()[:, :])
            nc.sync.dma_start(out=t_K[:, 0:8, :, :], in_=d_K.ap()[:, 0:8, :, :])
            nc.sync.dma_start(out=t_K[:, 8:12, :, :], in_=d_K.ap()[:, 8:12, :, :])
            nc.gpsimd.dma_start(out=t_d16[:, :, :], in_=d_data16.ap()[:, :, :])
            nc.gpsimd.dma_start(out=t_R8[:, :, 4:8, :],
                                in_=d_R8.ap()[:, :, 4:8, :])
            nc.gpsimd.dma_start(out=t_R8[:, :, 0:4, :],
                                in_=d_R8.ap()[:, :, 0:4, :])
            nc.gpsimd.dma_start(out=t_Rh[:, :, :, :], in_=d_Rh.ap()[:, :, :, :])
            nc.vector.tensor_copy(t_hs[:, 0, :],
                                  t_s16[:, O_H016:O_H016 + 16])

            for k in range(SEP_K):
                nc.vector.memset(t_We[k][:, 0:4, :, :], 0.0)

            # ---- prologue: data^T  [p, nc, b, t] ----
            # copies alternate DVE/ACT so the PE->copy pipeline runs at
            # ~half the single-engine cadence
            for b in range(BL):
                for n2 in range(2):
                    ps = bpool.tile([128, 128], f16, tag="bigps")
                    nc.tensor.transpose(ps[:, :],
                                        t_d16[:, b, 128 * n2:128 * (n2 + 1)],
                                        IDENT)
                    if (2 * b + n2) % 2 == 0:
                        nc.vector.tensor_copy(t_dT[:, n2, b, :], ps[:, :])
                    else:
                        nc.scalar.activation(t_dT[:, n2, b, :], ps[:, :],
                                             Act.Identity)

            # ---- prologue: 2*mx = data @ 2K (+2*biases), scattered per t.
            # Two t-passes so the first gates aren't stuck behind 12 full
            # 612ns scatter activations: t[0:32] now, t[32:128] in bg.
            def emit_mx(uc, t0, t1):
                ps = bpool.tile([128, BL, t1 - t0], f32, tag="bigps",
                                name=f"mx_{uc}_{t0}")
                for n2 in range(2):
                    nc.tensor.matmul(ps[:, :, :], t_K[:, uc, n2, :],
                                     t_dT[:, n2, :, t0:t1],
                                     start=(n2 == 0), stop=(n2 == 1))
                g, j = divmod(uc, 4)
                if g < 2:
                    nc.scalar.activation(
                        t_addmx[:, 4 * uc:4 * uc + 4, t0:t1], ps[:, :, :],
                        Act.Identity, bias=t_s32[:, uc:uc + 1])
                else:
                    # xh scatter on DVE (runs parallel with the z/r ACT ones)
                    nc.vector.tensor_scalar(
                        t_xh[:, 4 * j:4 * j + 4, t0:t1], ps[:, :, :],
                        t_s32[:, 8 + j:9 + j], None, Alu.add)

            for uc in range(12):
                emit_mx(uc, 0, 16)

            # s = data^T @ w1 + w1_b, emitted as bg thunks (DVE scatter in
            # windowA, matmul on the then-idle PE)
            def emit_s(b):
                ps = bpool.tile([128, N], f32, tag="bigps")
                nc.tensor.matmul(ps[:, :], W1, t_d16[:, b, :],
                                 start=True, stop=True)
                nc.vector.tensor_scalar(t_sxs[:, b, :], ps[:, :],
                                        t_s32[:, 12:13], None, Alu.add)

            # ---- lagged attention background ops ----
            def emit_hp_block(blk):
                t0 = 4 * blk
                ps = bpool.tile([128, 16], f32, tag="bigps", name=f"hpps_{blk}")
                for hc in range(4):
                    nc.tensor.matmul(ps[:, :], W2(hc),
                                     t_hs[:, t0 + 1:t0 + 5, 4 * hc:4 * hc + 4],
                                     start=(hc == 0), stop=(hc == 3))
                nc.vector.tensor_copy(t_hp[:, blk, :, :], ps[:, :])

            ts_tiles = {}

            def ts_tile(k):
                if k not in ts_tiles:
                    ts_tiles[k] = tspool.tile([128, BL, N], f16,
                                              tag=f"ts{k % 3}", name=f"ts_{k}")
                return ts_tiles[k]

            # D-build ops, sized so each step's bg load per engine stays
            # under ~600ns (ACT slack after the 3 gate ops, DVE slack after
            # the gate chain): base/ts are ACT; scale0/gD/X/d0f are DVE.
            def emit_base(q):
                nc.scalar.activation(t_X[:, q:q + 1, :], t_sxs[:, q:q + 1, :],
                                     Act.Tanh, scale=SEP_A0,
                                     bias=t_s32[:, 40:41])

            def emit_scale0(h):
                nc.vector.tensor_scalar(t_X[:, h:h + 2, :], t_X[:, h:h + 2, :],
                                        float(SEP_C0), None, Alu.mult)

            def emit_ts(k, q):
                nc.scalar.activation(ts_tile(k)[:, q:q + 1, :],
                                     t_sxs[:, q:q + 1, :],
                                     Act.Tanh, bias=t_s32[:, 26 + k:27 + k])

            g_tiles = {}

            def emit_g(k, h):
                g = wpool.tile([128, 2, N], f16, tag=f"g{h}", name=f"g_{k}_{h}")
                g_tiles[(k, h)] = g
                nc.vector.tensor_tensor(g[:, :, :], ts_tile(k)[:, h:h + 2, :],
                                        ts_tile(k + 1)[:, h:h + 2, :],
                                        Alu.subtract)

            def emit_D(k, h):
                g = g_tiles[(k, h)]
                nc.vector.tensor_scalar(t_D[k][:, h:h + 2, :], g[:, :, :],
                                        V_AP, float(SEP_C[k]),
                                        Alu.mult, Alu.mult)

            def emit_X(k, q):
                g = g_tiles[(k, 0 if q < 2 else 2)]
                nc.vector.scalar_tensor_tensor(
                    t_X[:, q:q + 1, :], g[:, q % 2, :], float(SEP_D[k]),
                    t_X[:, q:q + 1, :], Alu.mult, Alu.add)

            def emit_d0f(h):
                nc.vector.tensor_scalar(t_X[:, h:h + 2, :], t_X[:, h:h + 2, :],
                                        V_AP, 1.0, Alu.mult, Alu.mult)

            def emit_W_blocks(b0, b1, k):
                nc.scalar.activation(t_W[k][:, b0:b1, :, :],
                                     t_hp[:, b0:b1, :, :],
                                     Act.Tanh, scale=float(SEP_KAP[k]),
                                     bias=t_s32[:, 13 + k:14 + k])

            # Background schedule. Per scan step there are three usable idle
            # windows on the gate engines:
            #   bgACT[t]: ACT after cgN until next tr   (~480ns)
            #   bgA[t]:   DVE between mA and mB         (~435ns)
            #   bgB[t]:   DVE after hs until next mhc   (~330ns)
            # Each list holds at most one op sized under the window; ops are
            # pinned into the window with nosync ordering deps in the scan
            # loop. bgPE[t] holds PE/DMA work (no pinning needed).
            bgACT = {t: [] for t in range(T)}
            bgA = {t: [] for t in range(T)}
            bgB = {t: [] for t in range(T)}
            bgPE = {t: [] for t in range(T)}
            for uc in range(8):        # mx ACT scatters (372ns), two t-chunks
                bgACT[1 + uc].append(lambda uc=uc: emit_mx(uc, 16, 72))
                bgACT[9 + uc].append(lambda uc=uc: emit_mx(uc, 72, 128))
            for j, uc in enumerate(range(8, 12)):   # mx DVE scatters (293ns)
                bgA[1 + j].append(lambda uc=uc: emit_mx(uc, 16, 72))
                bgA[5 + j].append(lambda uc=uc: emit_mx(uc, 72, 128))
            for b in range(BL):        # s scatters (392ns DVE + PE matmul)
                bgA[9 + b].append(lambda b=b: emit_s(b))
            for q in range(4):         # base tanh quarters (398ns ACT)
                bgACT[17 + q].append(lambda q=q: emit_base(q))
            for i, h in enumerate((0, 2)):   # scale0 halves (327ns DVE)
                bgA[29 + i].append(lambda h=h: emit_scale0(h))
            for k in range(SEP_K + 1):       # ts quarters (398ns ACT)
                for q in range(4):
                    bgACT[23 + 6 * k + q].append(
                        lambda k=k, q=q: emit_ts(k, q))
            for k in range(SEP_K):
                # stride-6 template: heavy (~330ns) ops all in windowA; the
                # light D halves (194ns) ride windowB on non-hp steps (hp
                # steps' windowB holds only the 142ns hp copy).
                s = 33 + 6 * k
                bgA[s].append(lambda k=k: emit_g(k, 0))
                bgA[s + 1].append(lambda k=k: emit_g(k, 2))
                bgA[s + 2].append(lambda k=k: emit_X(k, 0))
                bgA[s + 3].append(lambda k=k: emit_X(k, 1))
                bgA[s + 4].append(lambda k=k: emit_X(k, 2))
                bgA[s + 5].append(lambda k=k: emit_X(k, 3))
                bslots = [x for x in range(s + 1, s + 6) if x % 4 != 0]
                bgB[bslots[0]].append(lambda k=k: emit_D(k, 0))
                bgB[bslots[1]].append(lambda k=k: emit_D(k, 2))
            for i, h in enumerate((0, 2)):
                bgA[111 + i].append(lambda h=h: emit_d0f(h))
            # W features (ACT): 8-block chunks on the ACT steps left free by
            # the 6-stride ts pattern; blocks 16:28 as one 12-block op
            # (345ns) once hp27 is ready; 28:32 in the epilogue.
            w23_slots = [70, 76, 82, 88, 94, 100, 106, 107, 108, 109, 110,
                         111, 112]
            for k in range(SEP_K):
                bgACT[33 + 6 * k].append(lambda k=k: emit_W_blocks(0, 8, k))
                bgACT[w23_slots[k]].append(
                    lambda k=k: emit_W_blocks(8, 16, k))
                bgACT[113 + k].append(lambda k=k: emit_W_blocks(16, 28, k))
            def emit_dout_dma():
                h = nc.sync.dma_start(out=t_dout[:, :, :],
                                      in_=d_dataout.ap()[:, :, :])
                # keep this 512KB transfer off the DMA engines until the
                # prologue's critical loads are done
                h.ins.bass_wait_until_ts = 60000
            bgPE[40].append(emit_dout_dma)

            # mid-scan score matmuls: const seeds + blocks 0:24 (96 t-rows)
            sc_tiles = {}

            def emit_score_seed():
                sc_tiles['01'] = scpool.tile([128, 2, N], f32, tag="sc01",
                                             name="scps01")
                sc_tiles['23'] = scpool.tile([128, 2, N], f32, tag="sc23",
                                             name="scps23")
                nc.tensor.matmul(sc_tiles['01'][:, :, :], ONES, t_X[:, 0:2, :],
                                 start=True, stop=False, skip_group_check=True)
                nc.tensor.matmul(sc_tiles['23'][:, :, :], ONES, t_X[:, 2:4, :],
                                 start=True, stop=False, skip_group_check=True)

            def emit_score_mm(k, r0, r1, b0, b1, stop_last=False, epi=False):
                # accumulate W[k](blocks b0:b1) @ D[k] into score rows r0:r1;
                # stop fires on the very last write of the whole group.
                for b in range(BL):
                    pst = sc_tiles['01' if b < 2 else '23']
                    col = b % 2
                    stat = (t_We[k][:, :, :, b] if epi
                            else t_W[k][:, b0:b1, :, b])
                    nc.tensor.matmul(pst[r0:r1, col, :],
                                     stat,
                                     t_D[k][:, b, :],
                                     start=False,
                                     stop=(stop_last and k == SEP_K - 1
                                           and col == 1),
                                     skip_group_check=True,
                                     tile_position=(0, r0))

            bgPE[113].append(emit_score_seed)
            for k in range(SEP_K):
                bgPE[114 + k].append(
                    lambda k=k: emit_score_mm(k, 0, 96, 0, 24))
                bgPE[115 + k].append(
                    lambda k=k: emit_score_mm(k, 96, 112, 24, 28))

            # ---- main scan loop ----
            prev_mA = prev_mB = None
            pendB, pendACT = [], []
            for t in range(T):
                rps = grpool.tile([128, 16], f32)
                zhps = gzpool.tile([128, 32], f32)  # cols: z 0:16 | h 16:32
                mhr = rps[:, 0:16]
                mhz, mhh = zhps[:, 0:16], zhps[:, 16:32]
                # seeds: r bank start=True; zh bank z-seed clears the bank,
                # h-seed first-touches its window (has_written cleared).
                nc.tensor.matmul(mhr, IDENT, t_addmx[:, 16:32, t],
                                 start=True, stop=False, skip_group_check=True)
                nc.tensor.matmul(mhz, IDENT, t_addmx[:, 0:16, t],
                                 start=True, stop=False, skip_group_check=True)
                nc.tensor.matmul(mhh, IDENT, BRECH,
                                 start=False, stop=False, skip_group_check=True)
                # h(t)@R arrives as two waves: A = 2z(.)h, B = 2(1-z)(.)c
                # (movings sum to 2h(t)); z/r stationaries unscaled, R_h/2.
                # wave-B starts with the r bank so sigmoid_r fires first;
                # z is last so the zh-tile's final write gates only zt/mhc
                # (both have slack until cgN).
                movings = ([H02] if t == 0
                           else [prev_mA[:, :], prev_mB[:, :]])
                for wi, mov in enumerate(movings):
                    lastw = wi == len(movings) - 1
                    for bankps, wsel in ((mhr, 'r'), (mhh, 'h'), (mhz, 'z')):
                        for uc in range(4):
                            for kc in range(4):
                                stat = (t_Rh[:, kc, uc, :] if wsel == 'h'
                                        else t_R8[:, kc, uc + (4 if wsel == 'r' else 0), :])
                                nc.tensor.matmul(
                                    bankps[:, 4 * uc:4 * uc + 4], stat,
                                    mov[:, 4 * kc:4 * kc + 4],
                                    start=False,
                                    stop=(lastw and wsel in ('r', 'z')
                                          and uc == 3 and kc == 3),
                                    skip_group_check=True)

                gate_prio = tc.high_priority(offset=200000)
                gate_prio.__enter__()

                def chain(h, want):
                    # record: instruction h.ins should carry its wait on the
                    # sem whose name starts with `want` (the critical-chain
                    # producer); other waits go to the spill EventSemaphore.
                    _CHAIN_FIX.append((h.ins.name, want))
                    return h

                tr = wpool.tile([128, 16], f16, tag="tr")
                mhc = wpool.tile([128, 16], f16, tag="mhc")
                t2 = wpool.tile([128, 16], f16, tag="t2")
                t3 = wpool.tile([128, 16], f16, tag="t3")
                cgN = wpool.tile([128, 16], f16, tag="cg")
                zt = wpool.tile([128, 16], f16, tag="zt")
                mA = wpool.tile([128, 16], f16, tag="mA")
                mB = wpool.tile([128, 16], f16, tag="mB")
                hsum = wpool.tile([128, 16], f16, tag="hsum")

                import bass_rust as _br

                def nsdep(inst, names):
                    s = _br.InstructionNameOrderedSet()
                    for nm in names:
                        s.add(nm)
                    if names:
                        inst.add_nosync_dependencies_from(s)

                mhc_h = nc.vector.tensor_copy(mhc[:, :], mhh)
                # pin prev step's windowB bg ops before this mhc
                nsdep(mhc_h.ins, pendB)
                # tr' = 2*sigma(r)-1, zt' = 2*sigma(z)-1
                tr_h = chain(nc.scalar.activation(tr[:, :], mhr, Act.Tanh,
                                                  scale=0.25), "PE")
                nsdep(tr_h.ins, pendACT)
                pendB, pendACT = [], []
                # t2 = (tr'+1)*mhh = 2*sigma_r*mhh ; t3 = t2 + 2*xh
                chain(nc.vector.scalar_tensor_tensor(t2[:, :], tr[:, :], 1.0,
                                                     mhc[:, :], Alu.add,
                                                     Alu.mult), "Activation")
                chain(nc.vector.tensor_tensor(t3[:, :], t2[:, :],
                                              t_xh[:, :, t], Alu.add), "DVE")
                nc.scalar.activation(zt[:, :], mhz, Act.Tanh,
                                     scale=0.25)
                # mA = (zt'+1)*h = 2*z*h   (next step's wave-A moving)
                mA_h = nc.vector.scalar_tensor_tensor(mA[:, :], zt[:, :], 1.0,
                                                      t_hs[:, t, :], Alu.add,
                                                      Alu.mult)
                # cgN = -c = tanh(-0.5*t3)
                cgN_h = chain(nc.scalar.activation(cgN[:, :], t3[:, :],
                                                   Act.Tanh, scale=-0.5),
                              "DVE")
                # mB = (zt'-1)*(-c) = 2*(1-z)*c  (wave-B moving)
                mB_h = chain(nc.vector.scalar_tensor_tensor(
                    mB[:, :], zt[:, :], 1.0, cgN[:, :], Alu.subtract,
                    Alu.mult), "Activation")
                # h(t+1) = (mA + mB)/2
                nc.vector.tensor_tensor(hsum[:, :], mA[:, :], mB[:, :],
                                        Alu.add)
                hs_h = nc.vector.tensor_scalar(t_hs[:, t + 1, :], hsum[:, :],
                                               0.5, None, Alu.mult)
                prev_mA, prev_mB = mA, mB
                gate_prio.__exit__(None, None, None)

                # Pin bg ops into their per-step idle windows via nosync
                # ordering deps (no sems): windowA = DVE between mA and mB;
                # windowB = DVE after hs (before next step's mhc); ACT ops
                # after cgN (before next step's tr).
                def run_window(thunks, eng_sfx, after_name):
                    got = []
                    for thunk in thunks:
                        n0 = len(nc.inst_map)
                        thunk()
                        for nm in list(nc.inst_map.keys())[n0:]:
                            ins = nc.inst_map[nm]
                            if str(ins.engine).endswith(eng_sfx):
                                nsdep(ins, [after_name])
                                got.append(nm)
                    return got

                if not SCAN_ONLY:
                    a_names = run_window(bgA.get(t, []), "DVE",
                                         mA_h.ins.name)
                    nsdep(mB_h.ins, a_names)
                    pendB = run_window(bgB.get(t, []), "DVE",
                                       hs_h.ins.name)
                    pendACT = run_window(bgACT.get(t, []), "Activation",
                                         cgN_h.ins.name)
                    for thunk in bgPE.get(t, []):
                        thunk()
                if t % 4 == 0 and t >= 4:
                    hp_names = run_window([lambda: emit_hp_block(t // 4 - 1)],
                                          "DVE", hs_h.ins.name)
                    nsdep(nc.inst_map[hp_names[0]], pendB)
                    pendB = pendB + hp_names

            # ---- attention epilogue ----
            def emit_out(b):
                pst = sc_tiles['01' if b < 2 else '23']
                col = b % 2
                ex = wpool.tile([128, N], f16, tag="expv", name=f"exp_{b}")
                nc.scalar.activation(ex[:, :], pst[:, col, :], Act.Exp,
                                     accum_out=t_ssum[:, b:b + 1])
                nc.vector.reciprocal(t_rinv[:, b:b + 1], t_ssum[:, b:b + 1])
                ot = wpool.tile([128, N], f32, tag="omul", name=f"out_{b}")
                nc.vector.scalar_tensor_tensor(
                    ot[:, :], ex[:, :], t_rinv[:, b:b + 1], t_dout[:, b, :],
                    Alu.mult, Alu.mult)
                nc.sync.dma_start(out=d_out.ap()[b, :, :], in_=ot[:, :])

            def emit_epi_mm(k, bs, stop_last):
                for b in bs:
                    pst = sc_tiles['01' if b < 2 else '23']
                    col = b % 2
                    nc.tensor.matmul(pst[96:128, col, :],
                                     t_We[k][:, :, :, b],
                                     t_D[k][:, b, :],
                                     start=False,
                                     stop=(stop_last and k == SEP_K - 1
                                           and col == 1),
                                     skip_group_check=True,
                                     tile_position=(0, 96))

            if not SCAN_ONLY:
                emit_hp_block(31)
                # pair 01 completes first so its softmax/output overlaps
                # pair 23's matmuls
                for k in range(SEP_K):
                    nc.scalar.activation(t_We[k][:, 4:8, :, :],
                                         t_hp[:, 28:32, :, :],
                                         Act.Tanh, scale=float(SEP_KAP[k]),
                                         bias=t_s32[:, 13 + k:14 + k])
                    emit_epi_mm(k, (0, 1), stop_last=True)
                emit_out(0)
                emit_out(1)
                for k in range(SEP_K):
                    emit_epi_mm(k, (2, 3), stop_last=True)
                emit_out(2)
                emit_out(3)
            else:
                emit_hp_block(31)
                nc.sync.dma_start(out=t_dout[:, :, :],
                                  in_=d_dataout.ap()[:, :, :])
                for b in range(BL):
                    ot = wpool.tile([128, N], f32, tag="omul", name=f"out_{b}")
                    nc.vector.tensor_copy(ot[:, :], t_dout[:, b, :])
                    nc.sync.dma_start(out=d_out.ap()[b, :, :], in_=ot[:, :])

            if DEBUG:
                nc.sync.dma_start(out=d_hs.ap()[:, :, :], in_=t_hs[:, :, :])
                nc.sync.dma_start(out=d_sxd.ap()[:, :, :], in_=t_sxs[:, :, :])
                nc.sync.dma_start(out=d_hpd.ap()[:, :, :, :],
                                  in_=t_hp[:, :, :, :])
                nc.sync.dma_start(out=d_d0d.ap()[:, :, :], in_=t_X[:, :, :])

    nc.compile()
    if os.environ.get("NN_ENC_NO_SWAP", "0") != "1":
        _sync_swap(nc)
    if os.environ.get("NN_ENC_ELIDE", "0") == "1":
        # NOTE: verified UNSAFE on real HW (NaN) — same-engine back-to-back
        # RAW needs the sem; kept only for experiments.
        _elide_same_engine_waits(nc)
    return nc


def _elide_same_engine_waits(nc):
    """Drop semaphore waits that are implied by same-engine program order.

    Engine queues execute in order and an op's memory write completes while
    the engine is busy (only the ack/sem visibility lags). So a wait on the
    engine's OWN counting semaphore, whose target count was already reached
    by increments from instructions earlier in this engine's stream, is
    redundant — remove it. Sems ever incremented by DMA-class instructions
    are excluded (DMA completion is asynchronous to queue order).
    """
    import concourse.mybir as mybir

    fn = nc.m.functions[0]
    dma_ops = {"DMACopy", "DMATrigger", "CollectiveCompute"}

    # pass 1: which sems are incremented by which engines / by DMAs
    sem_engines = {}
    sem_dma = set()
    for blk in fn.blocks:
        for i in blk.instructions:
            si = i.sync_info
            if si is None:
                continue
            for u in si.on_update:
                if u.sync_type != "semaphore":
                    continue
                sem_engines.setdefault(u.ant_name, set()).add(str(i.engine))
                if (i.opcode in dma_ops or "DMA" in i.opcode
                        or u.update_mode != "sem-inc"):
                    sem_dma.add(u.ant_name)

    allowed = set(os.environ.get("NN_ENC_ELIDE_ENG",
                                 "EngineType.DVE").split(","))
    nel = 0
    for blk in fn.blocks:
        counts = {}
        for i in blk.instructions:
            si = i.sync_info
            if si is None:
                continue
            waits = list(si.on_wait)
            if waits and str(i.engine) in allowed:
                eng = str(i.engine)
                keep = []
                for w in waits:
                    ok = (w.sync_type == "semaphore"
                          and w.wait_reg is None
                          and w.ant_name not in sem_dma
                          and sem_engines.get(w.ant_name) == {eng}
                          and w.wait_mode == "sem-ge-imm"
                          and counts.get(w.ant_name, 0) >= w.wait_value)
                    if ok:
                        nel += 1
                    else:
                        keep.append(w)
                if len(keep) != len(waits):
                    i.sync_info = mybir.SyncInfo(
                        on_wait=keep, on_update=list(si.on_update))
            for u in si.on_update:
                if u.sync_type == "semaphore" and u.update_mode == "sem-inc":
                    counts[u.ant_name] = counts.get(u.ant_name, 0) \
                        + u.update_value
    if DEBUG:
        print(f"_elide_same_engine_waits: {nel} waits elided")


def _sync_swap(nc):
    """Post-compile pass: when a spill EventSemaphore E carries the real
    (cross-engine) wait and its guarded instruction I carries only a trivial
    own-engine-counter wait, swap them. Both arrangements block I's execution
    until all waits are satisfied (E and I are adjacent on the same in-order
    queue), but with the chain wait attached to I, the sequencer can decode I
    while the wait is pending, hiding ~80ns of decode+dispatch latency per
    occurrence on the critical chain.
    """
    import concourse.mybir as mybir

    nswap = 0
    for blk in nc.m.functions[0].blocks:
        insts = list(blk.instructions)
        for n, e in enumerate(insts):
            if e.opcode != "EventSemaphore":
                continue
            se = e.sync_info
            if se is None or list(se.on_update):
                continue
            waits_e = list(se.on_wait)
            if not 1 <= len(waits_e) <= 2:
                continue
            if any(w.sync_type != "semaphore" or w.wait_reg is not None
                   for w in waits_e):
                continue
            eng = e.engine
            tgt = None
            for j in range(n + 1, min(n + 40, len(insts))):
                if insts[j].engine == eng:
                    if insts[j].opcode in ("EventSemaphore", "Ldweights"):
                        break
                    tgt = insts[j]
                    break
            if tgt is None:
                continue
            si = tgt.sync_info
            if si is None:
                continue
            waits_i = list(si.on_wait)
            if len(waits_i) != 1:
                continue
            w = waits_i[0]
            if w.sync_type != "semaphore" or w.wait_reg is not None:
                continue
            eng_name = str(eng).split(".")[-1]
            if not w.ant_name.startswith(eng_name):
                continue  # attached wait already the cross-engine chain wait
            if len(waits_e) == 1:
                mv, rest = waits_e[0], []
            else:
                # move the wait most likely to fire last onto I (prefer
                # non-PE: PE bank sems fire early in the step)
                ws = sorted(waits_e,
                            key=lambda x: 0 if x.ant_name.startswith("PE") else 1)
                rest, mv = [ws[0]], ws[1]
            e.sync_info = mybir.SyncInfo(on_wait=rest + [w], on_update=[])
            tgt.sync_info = mybir.SyncInfo(on_wait=[mv],
                                           on_update=list(si.on_update))
            nswap += 1

    # targeted pass: gate-chain instructions declare which producer's sem
    # must ride attached (resolved post-decode); all other waits move to the
    # spill EventSemaphore in front.
    want = dict(_CHAIN_FIX)
    nfix = 0
    for blk in nc.m.functions[0].blocks:
        insts = list(blk.instructions)
        pos = {i.name: n for n, i in enumerate(insts)}
        for name, pref in want.items():
            n = pos.get(name)
            if n is None:
                continue
            tgt = insts[n]
            si = tgt.sync_info
            if si is None:
                continue
            # nearest preceding same-engine EventSemaphore (the spill)
            spill = None
            for j in range(n - 1, max(-1, n - 30), -1):
                ij = insts[j]
                if ij.engine != tgt.engine:
                    continue
                if ij.opcode == "EventSemaphore":
                    sj = ij.sync_info
                    if sj is not None and not list(sj.on_update):
                        spill = ij
                break
            allw = list(si.on_wait) + (
                list(spill.sync_info.on_wait) if spill is not None else [])
            if any(w.sync_type != "semaphore" or w.wait_reg is not None
                   for w in allw):
                continue
            hit = [w for w in allw if w.ant_name.startswith(pref)]
            rest = [w for w in allw if not w.ant_name.startswith(pref)]
            if len(hit) != 1 or len(rest) > 2:
                continue
            if spill is None:
                continue  # single attached wait: nothing to rearrange
            tgt.sync_info = mybir.SyncInfo(on_wait=hit,
                                           on_update=list(si.on_update))
            spill.sync_info = mybir.SyncInfo(on_wait=rest, on_update=[])
            nfix += 1
    if DEBUG:
        print(f"_sync_swap: {nswap} swaps, {nfix} chain fixes")


def _prep_inputs(data, h0, gru_kernel, gru_rkernel, gru_bias,
                 w1_w, w1_b, w2_w, w2_b, v_w, v_b):
    f16 = np.float16
    f32 = np.float32

    import ml_dtypes
    R_all = np.ascontiguousarray(
        gru_rkernel.reshape(4, 128, 12, 128).transpose(1, 0, 2, 3))
    R8_l = R_all[:, :, 0:8, :].astype(ml_dtypes.float8_e4m3)
    Rh_l = (0.5 * R_all[:, :, 8:12, :]).astype(f16)
    K16 = (2.0 * gru_kernel).astype(f16)              # (256, 1536), doubled
    K_l = np.ascontiguousarray(
        K16.reshape(2, 128, 12, 128).transpose(1, 2, 0, 3))

    b_in, b_rec = gru_bias[0].astype(f32), gru_bias[1].astype(f32)

    s16 = np.zeros((128, S16_COLS), f16)
    s16[:, O_W1:O_W1 + 128] = w1_w.astype(f16)
    s16[:, O_W2:O_W2 + 512] = np.ascontiguousarray(
        w2_w.astype(f16).reshape(4, 128, 128).transpose(1, 0, 2)).reshape(128, 512)
    s16[:, O_ID:O_ID + 128] = np.eye(128, dtype=f16)
    s16[:, O_ONES:O_ONES + 128] = 1.0
    for j in range(4):
        for bb in range(4):
            s16[:, O_BRECH + 4 * j + bb] = b_rec[1024 + 128 * j:1024 + 128 * (j + 1)]

    s32 = np.zeros((128, S32_COLS), f32)
    s32[:, 0:8] = 2.0 * (b_in + b_rec)[:1024].reshape(8, 128).T
    s32[:, 8:12] = 2.0 * b_in[1024:].reshape(4, 128).T
    s32[:, 12] = w1_b.astype(f32)
    kap = np.asarray(SEP_KAP, np.float64)
    mu = np.asarray(SEP_MU, np.float64)
    s32[:, 13:26] = (kap[None, :] * w2_b.astype(np.float64)[:, None]
                     + mu[None, :]).astype(f32)
    s32[:, 26:41] = np.asarray(list(SEP_A) + [SEP_B0], f32)[None, :]
    s32[:, 41] = v_w[:, 0].astype(f32)

    tt = np.arange(T)
    per_core = []
    for c in range(NC):
        sl = slice(BL * c, BL * (c + 1))
        s16c = s16.copy()
        for j in range(4):
            for bb in range(BL):
                hcol = h0[BL * c + bb, 128 * j:128 * (j + 1)]
                s16c[:, O_H02 + 4 * j + bb] = 2.0 * hcol
                s16c[:, O_H016 + 4 * j + bb] = hcol
        # data16[t, l, n] = data[4c+l, t, n]
        data16 = np.ascontiguousarray(
            data[sl].astype(f16).transpose(1, 0, 2))
        # dataout[tt, l, n] = data[tt//4, 32*(tt%4) + 4c + l, n]
        dataout = np.empty((T, BL, N), f32)
        for l in range(BL):
            dataout[:, l, :] = data[tt // 4, 32 * (tt % 4) + 4 * c + l, :]
        per_core.append({
            "data16": data16, "dataout": dataout,
            "R8_l": R8_l, "Rh_l": Rh_l, "K_l": K_l,
            "small16": s16c, "small32": s32,
        })
    return per_core


def kernel(**inputs):
    from concourse.bass_utils import run_bass_kernel_spmd

    if "nc" not in _CACHE:
        _CACHE["nc"] = _build()
    nc = _CACHE["nc"]

    args = {k: np.asarray(v) for k, v in inputs.items()}
    per_core = _prep_inputs(
        args["data"], args["h0"], args["gru_kernel"], args["gru_rkernel"],
        args["gru_bias"], args["w1_w"], args["w1_b"], args["w2_w"],
        args["w2_b"], args["v_w"], args["v_b"])

    if "warm" not in _CACHE:
        # first execution after NEFF load can race; discard it
        run_bass_kernel_spmd(nc, per_core, core_ids=list(range(NC)))
        _CACHE["warm"] = True
    res = run_bass_kernel_spmd(nc, per_core, core_ids=list(range(NC)))
    _CACHE["last_res"] = res

    out = np.empty((B, T, N), np.float32)
    tt = np.arange(T)
    for c in range(NC):
        o = res.results[c]["out"]           # [BL, T(tt), N]
        for l in range(BL):
            out[tt // 4, 32 * (tt % 4) + 4 * c + l, :] = o[l]
    return out



# revision 48
# speedup vs baseline: 1.0099x; 1.0099x over previous
"""Trainium2 Bass kernel for nn_Encoder (GRU + input attention).

Shapes (hardcoded): B=32, T=128, N=256, H=512; 8 NeuronCores, batch
sharded 4 examples/core.

Math (matching the reference):
  hs = GRU scan over T steps (Keras GRUCell, reset_after=True, gates z,r,h)
  s[b,n,u]  = sum_t data[b,t,n] w1_w[t,u] + w1_b[u]
  h[t,b,u]  = hs[t,b,:] @ w2_w + w2_b[u]
  score[t,b,n] = sum_u v[u] tanh(s + h)   (+v_b: softmax-invariant)
  alpha = softmax_n(score);  out[b,t,:] = data[b,t,:] * alpha[...]

Key restructure vs the 317us baseline: tanh(s+h) is replaced by a
separable shifted-tanh model fitted offline (fixed universal constants):
  tanh(s+h) ~= c0*tanh(a0*s+b0) + d0
             + sum_k [tanh(s+A_k)-tanh(s+A_{k+1})] * (c_k*tanh(kap_k*h+mu_k) + d_k)
so score becomes K+1=14 PE matmuls per example (stationary = tanh
features of h laid [u,t], moving = v-weighted s-features [u,n]) instead
of a 134M-element e=tanh tensor. Removes ~1.2us/step of Activation work
and all per-step e-adds/score matmuls from the scan steady state.
End-to-end error (incl f16 + fp8 GRU weights): ~6.7e-3 rel.

GRU scan structure per step (latency-optimized):
 - h(t+1)@R is evaluated as two matmul waves (z(.)h)@R + ((1-z)(.)c)@R so
   wave A runs during the r/c gate chain and only wave B (16 r-bank
   matmuls) gates sigmoid_r of the next step; the h-assembly is off the
   critical path.
 - All gate nonlinearities are Tanh (sigma(x)=(1+tanh(x/2))/2 folded via
   host-side scaling: K/input-biases doubled, R_h halved, ACT input
   scales 0.25/0.5) so one activation table set (exp_and_others:
   Tanh+Exp) serves the whole kernel - zero table swaps.
 - wave movings mA=(zt'+1)(.)h=2zh and mB=(zt'-1)(.)(-c)=2(1-z)c are one
   scalar_tensor_tensor op each; cgN=-c comes free via ACT scale=-0.5.
Attention background (s-features, D tensors) threads into scan idle gaps
at 1 op per 2 steps; hp blocks stay lagged 1 block behind the scan; the
W_k=tanh(kap_k*hp+..) features + 14 score matmuls + softmax + final
multiply run in a pipelined epilogue. Small constants ride in two
batched DMA blobs to cut HWDGE serialization in the prologue.
"""

import os
import sys

import numpy as np

# concourse (Bass) lives in the TRN2 container; make sure it's importable
for _p in ("/root/.axon_site", "/root/.axon_site/_ro/trn_rl_repo",
           "/root/.axon_site/_ro/pypackages", "/opt/trn_rl_repo",
           "/opt/pypackages"):
    if os.path.isdir(_p) and _p not in sys.path:
        sys.path.append(_p)

B, T, N, H = 32, 128, 256, 512
NC = 8           # cores
BL = B // NC     # batch per core (4)
H3 = 3 * H

_CACHE = {}
_CHAIN_FIX = []
DEBUG = False
SCAN_ONLY = os.environ.get("NN_ENC_SCAN_ONLY", "0") == "1"

# ---- separable tanh(s+h) model constants (fitted offline; universal) ----
SEP_K = 13
SEP_A = [-3.2709594, -2.5955656, -1.9867907, -1.3730230, -0.8524583,
         -0.4658397, -0.1405748, 0.1436354, 0.4704521, 0.8618943,
         1.3933731, 2.0088742, 2.6093273, 3.3204157]
SEP_C = [-0.5496367, -0.5309651, -0.5037420, -0.5256692, -0.4968104,
         -0.5381408, -0.5060670, -0.5380091, -0.4986397, -0.5247371,
         -0.5046583, -0.5389031, -0.5067456]
SEP_D = [0.5366726, 0.5316761, 0.5094128, 0.5085504, 0.4964807,
         0.4820233, 0.5037248, 0.5113558, 0.5045732, 0.4939719,
         0.4838700, 0.4722759, 0.5153870]
SEP_KAP = [3.1443172, 3.3217070, 3.4944437, 3.7533102, 4.4334431,
           4.6020584, 5.0746264, 4.5910144, 4.3981624, 3.7172728,
           3.4599097, 3.2798653, 3.1275754]
SEP_MU = [9.3775816, 7.7132607, 5.9568019, 4.2126617, 2.9646783,
          1.4329745, -0.0097532, -1.4501734, -2.9762077, -4.2356744,
          -5.9752331, -7.6982388, -9.3694906]
SEP_C0 = 0.9994611
SEP_A0 = 1.0115169
SEP_B0 = 3.3683648

# smallf16 blob column offsets
O_W1, O_W2, O_ID, O_ONES, O_BRECH, O_H02, O_H016 = 0, 128, 640, 768, 896, 912, 928
S16_COLS = 944
# smallf32 blob column offsets: bzr 0:8, bh 8:12, w1b 12, bW 13:26, bA 26:41, v 41
S32_COLS = 42


def _build():
    import concourse.bass as bass
    import concourse.bacc as bacc
    import concourse.tile as tile
    import concourse.mybir as mybir

    f16 = mybir.dt.float16
    f32 = mybir.dt.float32
    Alu = mybir.AluOpType
    Act = mybir.ActivationFunctionType

    nc = bacc.Bacc("TRN2", target_bir_lowering=False, debug=False)

    # ---- dram I/O ----
    d_data16 = nc.dram_tensor("data16", [T, BL, N], f16, kind="ExternalInput")
    d_dataout = nc.dram_tensor("dataout", [T, BL, N], f32, kind="ExternalInput")
    f8 = mybir.dt.float8e4
    d_R8 = nc.dram_tensor("R8_l", [128, 4, 8, 128], f8, kind="ExternalInput")
    d_Rh = nc.dram_tensor("Rh_l", [128, 4, 4, 128], f16, kind="ExternalInput")
    d_K = nc.dram_tensor("K_l", [128, 12, 2, 128], f16, kind="ExternalInput")
    d_s16 = nc.dram_tensor("small16", [128, S16_COLS], f16, kind="ExternalInput")
    d_s32 = nc.dram_tensor("small32", [128, S32_COLS], f32, kind="ExternalInput")
    d_out = nc.dram_tensor("out", [BL, T, N], f32, kind="ExternalOutput")
    if DEBUG:
        d_hs = nc.dram_tensor("hs_dump", [128, T + 1, 16], f16,
                              kind="ExternalOutput")
        d_sxd = nc.dram_tensor("sx_dump", [128, BL, N], f16,
                               kind="ExternalOutput")
        d_hpd = nc.dram_tensor("hp_dump", [128, 32, 4, 4], f16,
                               kind="ExternalOutput")
        d_d0d = nc.dram_tensor("d0_dump", [128, BL, N], f16,
                               kind="ExternalOutput")

    with tile.TileContext(nc) as tc:
        with (
            tc.tile_pool(name="const", bufs=1) as cpool,
            tc.tile_pool(name="work", bufs=4) as wpool,
            tc.tile_pool(name="tsbuf", bufs=3) as tspool,
            tc.tile_pool(name="gater", bufs=2, space="PSUM") as grpool,
            tc.tile_pool(name="gatezh", bufs=2, space="PSUM") as gzpool,
            tc.tile_pool(name="bigps", bufs=2, space="PSUM") as bpool,
            tc.tile_pool(name="score", bufs=1, space="PSUM") as scpool,
        ):
            # ---- persistent tiles ----
            t_R8 = cpool.tile([128, 4, 8, 128], f8)
            t_Rh = cpool.tile([128, 4, 4, 128], f16)
            t_K = cpool.tile([128, 12, 2, 128], f16)
            t_s16 = cpool.tile([128, S16_COLS], f16)
            t_s32 = cpool.tile([128, S32_COLS], f32)
            t_d16 = cpool.tile([128, BL, N], f16)          # data [t, b, n]
            t_dout = cpool.tile([128, BL, N], f32)         # dataout [tt, b, n]
            t_dT = cpool.tile([128, 2, BL, 128], f16)      # dataT [p, nc, b, t]
            t_addmx = cpool.tile([128, 32, T], f16)        # 2*mx_z' | 2*mx_r'
            t_xh = cpool.tile([128, 16, T], f16)           # 2*xh' per t
            t_sxs = cpool.tile([128, BL, N], f16)          # s = sx + w1_b
            t_hs = cpool.tile([128, T + 1, 16], f16)       # h^T packed
            t_hp = cpool.tile([128, 32, 4, 4], f16)        # hp [u, blk, tl, b]
            t_W = [cpool.tile([128, 32, 4, 4], f16, tag=f"W_{k}",
                              name=f"W_{k}") for k in range(SEP_K)]
            # epilogue stationary pad: [zeros(4 blks) | W blocks 28:32] so a
            # 32-wide stationary at tile_position row 96 adds only rows
            # 112:128 (PE tile_position rows are restricted to multiples
            # of 32).
            t_We = [cpool.tile([128, 8, 4, 4], f16, tag=f"We_{k}",
                               name=f"We_{k}") for k in range(SEP_K)]
            t_D = [cpool.tile([128, BL, N], f16, tag=f"D_{k}",
                              name=f"D_{k}") for k in range(SEP_K)]
            t_X = cpool.tile([128, BL, N], f16)            # D0 accumulator
            t_ssum = cpool.tile([128, BL], f32)
            t_rinv = cpool.tile([128, BL], f32)

            W1 = t_s16[:, O_W1:O_W1 + 128]
            IDENT = t_s16[:, O_ID:O_ID + 128]
            ONES = t_s16[:, O_ONES:O_ONES + 128]
            BRECH = t_s16[:, O_BRECH:O_BRECH + 16]
            H02 = t_s16[:, O_H02:O_H02 + 16]

            def W2(hc):
                return t_s16[:, O_W2 + 128 * hc:O_W2 + 128 * (hc + 1)]

            V_AP = t_s32[:, 41:42]

            # ---- DMA in, all on the hardware DGE queues (the Pool SWDGE
            # pays ~1us of descriptor generation per DMA, serialized on the
            # Pool engine). Issue order sets transfer priority: the scan
            # start is gated by d16 (-> transposes -> mx) and R8's r-chunks
            # (first wave's r bank); K2/Rh/R8z matter a few hundred ns later.
            nc.sync.dma_start(out=t_s16[:, :], in_=d_s16.ap()[:, :])
            nc.sync.dma_start(out=t_d16[:, :, :], in_=d_data16.ap()[:, :, :])
            nc.sync.dma_start(out=t_s32[:, :], in_=d_s32.ap()[:, :])
            nc.sync.dma_start(out=t_K[:, 0:8, :, :], in_=d_K.ap()[:, 0:8, :, :])
            nc.sync.dma_start(out=t_R8[:, :, 4:8, :],
                              in_=d_R8.ap()[:, :, 4:8, :])
            nc.sync.dma_start(out=t_K[:, 8:12, :, :],
                              in_=d_K.ap()[:, 8:12, :, :])
            nc.sync.dma_start(out=t_Rh[:, :, :, :], in_=d_Rh.ap()[:, :, :, :])
            nc.sync.dma_start(out=t_R8[:, :, 0:4, :],
                              in_=d_R8.ap()[:, :, 0:4, :])
            nc.vector.tensor_copy(t_hs[:, 0, :],
                                  t_s16[:, O_H016:O_H016 + 16])

            for k in range(SEP_K):
                nc.vector.memset(t_We[k][:, 0:4, :, :], 0.0)

            # ---- prologue: data^T  [p, nc, b, t] ----
            # copies alternate DVE/ACT so the PE->copy pipeline runs at
            # ~half the single-engine cadence
            for b in range(BL):
                for n2 in range(2):
                    ps = bpool.tile([128, 128], f16, tag="bigps")
                    nc.tensor.transpose(ps[:, :],
                                        t_d16[:, b, 128 * n2:128 * (n2 + 1)],
                                        IDENT)
                    nc.vector.tensor_copy(t_dT[:, n2, b, :], ps[:, :])

            # ---- prologue: 2*mx = data @ 2K (+2*biases), scattered per t.
            # Two t-passes so the first gates aren't stuck behind 12 full
            # 612ns scatter activations: t[0:32] now, t[32:128] in bg.
            def emit_mx(uc, t0, t1):
                ps = bpool.tile([128, BL, t1 - t0], f32, tag="bigps",
                                name=f"mx_{uc}_{t0}")
                for n2 in range(2):
                    nc.tensor.matmul(ps[:, :, :], t_K[:, uc, n2, :],
                                     t_dT[:, n2, :, t0:t1],
                                     start=(n2 == 0), stop=(n2 == 1))
                g, j = divmod(uc, 4)
                if g < 2:
                    nc.scalar.activation(
                        t_addmx[:, 4 * uc:4 * uc + 4, t0:t1], ps[:, :, :],
                        Act.Identity, bias=t_s32[:, uc:uc + 1])
                else:
                    # xh scatter on DVE (runs parallel with the z/r ACT ones)
                    nc.vector.tensor_scalar(
                        t_xh[:, 4 * j:4 * j + 4, t0:t1], ps[:, :, :],
                        t_s32[:, 8 + j:9 + j], None, Alu.add)

            for uc in range(12):
                emit_mx(uc, 0, 16)

            # s = data^T @ w1 + w1_b, emitted as bg thunks (DVE scatter in
            # windowA, matmul on the then-idle PE)
            def emit_s(b):
                ps = bpool.tile([128, N], f32, tag="bigps")
                nc.tensor.matmul(ps[:, :], W1, t_d16[:, b, :],
                                 start=True, stop=True)
                nc.vector.tensor_scalar(t_sxs[:, b, :], ps[:, :],
                                        t_s32[:, 12:13], None, Alu.add)

            # ---- lagged attention background ops ----
            def emit_hp_block(blk):
                t0 = 4 * blk
                ps = bpool.tile([128, 16], f32, tag="bigps", name=f"hpps_{blk}")
                for hc in range(4):
                    nc.tensor.matmul(ps[:, :], W2(hc),
                                     t_hs[:, t0 + 1:t0 + 5, 4 * hc:4 * hc + 4],
                                     start=(hc == 0), stop=(hc == 3))
                nc.vector.tensor_copy(t_hp[:, blk, :, :], ps[:, :])

            ts_tiles = {}

            def ts_tile(k):
                if k not in ts_tiles:
                    ts_tiles[k] = tspool.tile([128, BL, N], f16,
                                              tag=f"ts{k % 3}", name=f"ts_{k}")
                return ts_tiles[k]

            # D-build ops, sized so each step's bg load per engine stays
            # under ~600ns (ACT slack after the 3 gate ops, DVE slack after
            # the gate chain): base/ts are ACT; scale0/gD/X/d0f are DVE.
            def emit_base(q):
                nc.scalar.activation(t_X[:, q:q + 1, :], t_sxs[:, q:q + 1, :],
                                     Act.Tanh, scale=SEP_A0,
                                     bias=t_s32[:, 40:41])

            def emit_scale0(h):
                nc.vector.tensor_scalar(t_X[:, h:h + 2, :], t_X[:, h:h + 2, :],
                                        float(SEP_C0), None, Alu.mult)

            def emit_ts(k, q):
                nc.scalar.activation(ts_tile(k)[:, q:q + 1, :],
                                     t_sxs[:, q:q + 1, :],
                                     Act.Tanh, bias=t_s32[:, 26 + k:27 + k])

            g_tiles = {}

            def emit_g(k, h):
                g = wpool.tile([128, 2, N], f16, tag=f"g{h}", name=f"g_{k}_{h}")
                g_tiles[(k, h)] = g
                nc.vector.tensor_tensor(g[:, :, :], ts_tile(k)[:, h:h + 2, :],
                                        ts_tile(k + 1)[:, h:h + 2, :],
                                        Alu.subtract)

            def emit_D(k, h):
                g = g_tiles[(k, h)]
                nc.vector.tensor_scalar(t_D[k][:, h:h + 2, :], g[:, :, :],
                                        V_AP, float(SEP_C[k]),
                                        Alu.mult, Alu.mult)

            def emit_X(k, q):
                g = g_tiles[(k, 0 if q < 2 else 2)]
                nc.vector.scalar_tensor_tensor(
                    t_X[:, q:q + 1, :], g[:, q % 2, :], float(SEP_D[k]),
                    t_X[:, q:q + 1, :], Alu.mult, Alu.add)

            def emit_d0f(h):
                nc.vector.tensor_scalar(t_X[:, h:h + 2, :], t_X[:, h:h + 2, :],
                                        V_AP, 1.0, Alu.mult, Alu.mult)

            def emit_W_blocks(b0, b1, k):
                nc.scalar.activation(t_W[k][:, b0:b1, :, :],
                                     t_hp[:, b0:b1, :, :],
                                     Act.Tanh, scale=float(SEP_KAP[k]),
                                     bias=t_s32[:, 13 + k:14 + k])

            # Background schedule. Per scan step there are three usable idle
            # windows on the gate engines:
            #   bgACT[t]: ACT after cgN until next tr   (~480ns)
            #   bgA[t]:   DVE between mA and mB         (~435ns)
            #   bgB[t]:   DVE after hs until next mhc   (~330ns)
            # Each list holds at most one op sized under the window; ops are
            # pinned into the window with nosync ordering deps in the scan
            # loop. bgPE[t] holds PE/DMA work (no pinning needed).
            bgACT = {t: [] for t in range(T)}
            bgA = {t: [] for t in range(T)}
            bgB = {t: [] for t in range(T)}
            bgPE = {t: [] for t in range(T)}
            for uc in range(8):        # mx ACT scatters (372ns), two t-chunks
                bgACT[1 + uc].append(lambda uc=uc: emit_mx(uc, 16, 72))
                bgACT[9 + uc].append(lambda uc=uc: emit_mx(uc, 72, 128))
            for j, uc in enumerate(range(8, 12)):   # mx DVE scatters (293ns)
                bgA[1 + j].append(lambda uc=uc: emit_mx(uc, 16, 72))
                bgA[5 + j].append(lambda uc=uc: emit_mx(uc, 72, 128))
            for b in range(BL):        # s scatters (392ns DVE + PE matmul)
                bgA[9 + b].append(lambda b=b: emit_s(b))
            for q in range(4):         # base tanh quarters (398ns ACT)
                bgACT[17 + q].append(lambda q=q: emit_base(q))
            for i, h in enumerate((0, 2)):   # scale0 halves (327ns DVE)
                bgA[29 + i].append(lambda h=h: emit_scale0(h))
            for k in range(SEP_K + 1):       # ts quarters (398ns ACT)
                for q in range(4):
                    bgACT[23 + 6 * k + q].append(
                        lambda k=k, q=q: emit_ts(k, q))
            for k in range(SEP_K):
                # stride-6 template: heavy (~330ns) ops all in windowA; the
                # light D halves (194ns) ride windowB on non-hp steps (hp
                # steps' windowB holds only the 142ns hp copy).
                s = 33 + 6 * k
                bgA[s].append(lambda k=k: emit_g(k, 0))
                bgA[s + 1].append(lambda k=k: emit_g(k, 2))
                bgA[s + 2].append(lambda k=k: emit_X(k, 0))
                bgA[s + 3].append(lambda k=k: emit_X(k, 1))
                bgA[s + 4].append(lambda k=k: emit_X(k, 2))
                bgA[s + 5].append(lambda k=k: emit_X(k, 3))
                bslots = [x for x in range(s + 1, s + 6) if x % 4 != 0]
                bgB[bslots[0]].append(lambda k=k: emit_D(k, 0))
                bgB[bslots[1]].append(lambda k=k: emit_D(k, 2))
            for i, h in enumerate((0, 2)):
                bgA[111 + i].append(lambda h=h: emit_d0f(h))
            # W features (ACT): 8-block chunks on the ACT steps left free by
            # the 6-stride ts pattern; blocks 16:28 as one 12-block op
            # (345ns) once hp27 is ready; 28:32 in the epilogue.
            w23_slots = [70, 76, 82, 88, 94, 100, 106, 107, 108, 109, 110,
                         111, 112]
            for k in range(SEP_K):
                bgACT[33 + 6 * k].append(lambda k=k: emit_W_blocks(0, 8, k))
                bgACT[w23_slots[k]].append(
                    lambda k=k: emit_W_blocks(8, 16, k))
                bgACT[113 + k].append(lambda k=k: emit_W_blocks(16, 28, k))
            def emit_dout_hold():
                # fake reader of t_dout: the dout DMA (emitted next step)
                # then carries a WAR wait on this op, keeping its 512KB
                # transfer off the DMA engines until mid-scan (it would
                # otherwise be hoisted into the prologue and starve the
                # critical input loads).
                scratch = wpool.tile([128, 1], f32, tag="dhold")
                nc.vector.tensor_copy(scratch[:, :], t_dout[:, 0, 0:1])

            bgB[40].append(emit_dout_hold)
            bgPE[41].append(lambda: nc.sync.dma_start(
                out=t_dout[:, :, :], in_=d_dataout.ap()[:, :, :]))

            # mid-scan score matmuls: const seeds + blocks 0:24 (96 t-rows)
            sc_tiles = {}

            def emit_score_seed():
                sc_tiles['01'] = scpool.tile([128, 2, N], f32, tag="sc01",
                                             name="scps01")
                sc_tiles['23'] = scpool.tile([128, 2, N], f32, tag="sc23",
                                             name="scps23")
                nc.tensor.matmul(sc_tiles['01'][:, :, :], ONES, t_X[:, 0:2, :],
                                 start=True, stop=False, skip_group_check=True)
                nc.tensor.matmul(sc_tiles['23'][:, :, :], ONES, t_X[:, 2:4, :],
                                 start=True, stop=False, skip_group_check=True)

            def emit_score_mm(k, r0, r1, b0, b1, stop_last=False, epi=False):
                # accumulate W[k](blocks b0:b1) @ D[k] into score rows r0:r1;
                # stop fires on the very last write of the whole group.
                for b in range(BL):
                    pst = sc_tiles['01' if b < 2 else '23']
                    col = b % 2
                    stat = (t_We[k][:, :, :, b] if epi
                            else t_W[k][:, b0:b1, :, b])
                    nc.tensor.matmul(pst[r0:r1, col, :],
                                     stat,
                                     t_D[k][:, b, :],
                                     start=False,
                                     stop=(stop_last and k == SEP_K - 1
                                           and col == 1),
                                     skip_group_check=True,
                                     tile_position=(0, r0))

            bgPE[113].append(emit_score_seed)
            for k in range(SEP_K):
                bgPE[114 + k].append(
                    lambda k=k: emit_score_mm(k, 0, 96, 0, 24))
                bgPE[115 + k].append(
                    lambda k=k: emit_score_mm(k, 96, 112, 24, 28))

            # ---- main scan loop ----
            prev_mA = prev_mB = None
            pendB, pendACT = [], []
            for t in range(T):
                rps = grpool.tile([128, 16], f32)
                zhps = gzpool.tile([128, 32], f32)  # cols: z 0:16 | h 16:32
                mhr = rps[:, 0:16]
                mhz, mhh = zhps[:, 0:16], zhps[:, 16:32]
                # seeds: r bank start=True; zh bank z-seed clears the bank,
                # h-seed first-touches its window (has_written cleared).
                nc.tensor.matmul(mhr, IDENT, t_addmx[:, 16:32, t],
                                 start=True, stop=False, skip_group_check=True)
                nc.tensor.matmul(mhz, IDENT, t_addmx[:, 0:16, t],
                                 start=True, stop=False, skip_group_check=True)
                nc.tensor.matmul(mhh, IDENT, BRECH,
                                 start=False, stop=False, skip_group_check=True)
                # h(t)@R arrives as two waves: A = 2z(.)h, B = 2(1-z)(.)c
                # (movings sum to 2h(t)); z/r stationaries unscaled, R_h/2.
                # wave-B starts with the r bank so sigmoid_r fires first;
                # z is last so the zh-tile's final write gates only zt/mhc
                # (both have slack until cgN).
                movings = ([H02] if t == 0
                           else [prev_mA[:, :], prev_mB[:, :]])
                for wi, mov in enumerate(movings):
                    lastw = wi == len(movings) - 1
                    for bankps, wsel in ((mhr, 'r'), (mhh, 'h'), (mhz, 'z')):
                        for uc in range(4):
                            for kc in range(4):
                                stat = (t_Rh[:, kc, uc, :] if wsel == 'h'
                                        else t_R8[:, kc, uc + (4 if wsel == 'r' else 0), :])
                                nc.tensor.matmul(
                                    bankps[:, 4 * uc:4 * uc + 4], stat,
                                    mov[:, 4 * kc:4 * kc + 4],
                                    start=False,
                                    stop=(lastw and wsel in ('r', 'z')
                                          and uc == 3 and kc == 3),
                                    skip_group_check=True)

                gate_prio = tc.high_priority(offset=200000)
                gate_prio.__enter__()

                def chain(h, want):
                    # record: instruction h.ins should carry its wait on the
                    # sem whose name starts with `want` (the critical-chain
                    # producer); other waits go to the spill EventSemaphore.
                    _CHAIN_FIX.append((h.ins.name, want))
                    return h

                tr = wpool.tile([128, 16], f16, tag="tr")
                mhc = wpool.tile([128, 16], f16, tag="mhc")
                t2 = wpool.tile([128, 16], f16, tag="t2")
                t3 = wpool.tile([128, 16], f16, tag="t3")
                cgN = wpool.tile([128, 16], f16, tag="cg")
                zt = wpool.tile([128, 16], f16, tag="zt")
                mA = wpool.tile([128, 16], f16, tag="mA")
                mB = wpool.tile([128, 16], f16, tag="mB")
                hsum = wpool.tile([128, 16], f16, tag="hsum")

                import bass_rust as _br

                def nsdep(inst, names):
                    s = _br.InstructionNameOrderedSet()
                    for nm in names:
                        s.add(nm)
                    if names:
                        inst.add_nosync_dependencies_from(s)

                mhc_h = nc.vector.tensor_copy(mhc[:, :], mhh)
                # pin prev step's windowB bg ops before this mhc
                nsdep(mhc_h.ins, pendB)
                # tr' = 2*sigma(r)-1, zt' = 2*sigma(z)-1
                tr_h = chain(nc.scalar.activation(tr[:, :], mhr, Act.Tanh,
                                                  scale=0.25), "PE")
                nsdep(tr_h.ins, pendACT)
                pendB, pendACT = [], []
                # t2 = (tr'+1)*mhh = 2*sigma_r*mhh ; t3 = t2 + 2*xh
                chain(nc.vector.scalar_tensor_tensor(t2[:, :], tr[:, :], 1.0,
                                                     mhc[:, :], Alu.add,
                                                     Alu.mult), "Activation")
                chain(nc.vector.tensor_tensor(t3[:, :], t2[:, :],
                                              t_xh[:, :, t], Alu.add), "DVE")
                nc.scalar.activation(zt[:, :], mhz, Act.Tanh,
                                     scale=0.25)
                # mA = (zt'+1)*h = 2*z*h   (next step's wave-A moving)
                mA_h = nc.vector.scalar_tensor_tensor(mA[:, :], zt[:, :], 1.0,
                                                      t_hs[:, t, :], Alu.add,
                                                      Alu.mult)
                # cgN = -c = tanh(-0.5*t3)
                cgN_h = chain(nc.scalar.activation(cgN[:, :], t3[:, :],
                                                   Act.Tanh, scale=-0.5),
                              "DVE")
                # mB = (zt'-1)*(-c) = 2*(1-z)*c  (wave-B moving)
                mB_h = chain(nc.vector.scalar_tensor_tensor(
                    mB[:, :], zt[:, :], 1.0, cgN[:, :], Alu.subtract,
                    Alu.mult), "Activation")
                # h(t+1) = (mA + mB)/2
                nc.vector.tensor_tensor(hsum[:, :], mA[:, :], mB[:, :],
                                        Alu.add)
                hs_h = nc.vector.tensor_scalar(t_hs[:, t + 1, :], hsum[:, :],
                                               0.5, None, Alu.mult)
                prev_mA, prev_mB = mA, mB
                gate_prio.__exit__(None, None, None)

                # Pin bg ops into their per-step idle windows via nosync
                # ordering deps (no sems): windowA = DVE between mA and mB;
                # windowB = DVE after hs (before next step's mhc); ACT ops
                # after cgN (before next step's tr).
                def run_window(thunks, eng_sfx, after_name):
                    got = []
                    for thunk in thunks:
                        n0 = len(nc.inst_map)
                        thunk()
                        for nm in list(nc.inst_map.keys())[n0:]:
                            ins = nc.inst_map[nm]
                            if str(ins.engine).endswith(eng_sfx):
                                nsdep(ins, [after_name])
                                got.append(nm)
                    return got

                if not SCAN_ONLY:
                    a_names = run_window(bgA.get(t, []), "DVE",
                                         mA_h.ins.name)
                    nsdep(mB_h.ins, a_names)
                    pendB = run_window(bgB.get(t, []), "DVE",
                                       hs_h.ins.name)
                    pendACT = run_window(bgACT.get(t, []), "Activation",
                                         cgN_h.ins.name)
                    for thunk in bgPE.get(t, []):
                        thunk()
                if t % 4 == 0 and t >= 4:
                    hp_names = run_window([lambda: emit_hp_block(t // 4 - 1)],
                                          "DVE", hs_h.ins.name)
                    nsdep(nc.inst_map[hp_names[0]], pendB)
                    pendB = pendB + hp_names

            # ---- attention epilogue ----
            def emit_out(b):
                pst = sc_tiles['01' if b < 2 else '23']
                col = b % 2
                ex = wpool.tile([128, N], f16, tag="expv", name=f"exp_{b}")
                nc.scalar.activation(ex[:, :], pst[:, col, :], Act.Exp,
                                     accum_out=t_ssum[:, b:b + 1])
                nc.vector.reciprocal(t_rinv[:, b:b + 1], t_ssum[:, b:b + 1])
                ot = wpool.tile([128, N], f32, tag="omul", name=f"out_{b}")
                nc.vector.scalar_tensor_tensor(
                    ot[:, :], ex[:, :], t_rinv[:, b:b + 1], t_dout[:, b, :],
                    Alu.mult, Alu.mult)
                nc.sync.dma_start(out=d_out.ap()[b, :, :], in_=ot[:, :])

            def emit_epi_mm(k, bs, stop_last):
                for b in bs:
                    pst = sc_tiles['01' if b < 2 else '23']
                    col = b % 2
                    nc.tensor.matmul(pst[96:128, col, :],
                                     t_We[k][:, :, :, b],
                                     t_D[k][:, b, :],
                                     start=False,
                                     stop=(stop_last and k == SEP_K - 1
                                           and col == 1),
                                     skip_group_check=True,
                                     tile_position=(0, 96))

            if not SCAN_ONLY:
                emit_hp_block(31)
                # pair 01 completes first so its softmax/output overlaps
                # pair 23's matmuls
                for k in range(SEP_K):
                    nc.scalar.activation(t_We[k][:, 4:8, :, :],
                                         t_hp[:, 28:32, :, :],
                                         Act.Tanh, scale=float(SEP_KAP[k]),
                                         bias=t_s32[:, 13 + k:14 + k])
                    emit_epi_mm(k, (0, 1), stop_last=True)
                emit_out(0)
                emit_out(1)
                for k in range(SEP_K):
                    emit_epi_mm(k, (2, 3), stop_last=True)
                emit_out(2)
                emit_out(3)
            else:
                emit_hp_block(31)
                nc.sync.dma_start(out=t_dout[:, :, :],
                                  in_=d_dataout.ap()[:, :, :])
                for b in range(BL):
                    ot = wpool.tile([128, N], f32, tag="omul", name=f"out_{b}")
                    nc.vector.tensor_copy(ot[:, :], t_dout[:, b, :])
                    nc.sync.dma_start(out=d_out.ap()[b, :, :], in_=ot[:, :])

            if DEBUG:
                nc.sync.dma_start(out=d_hs.ap()[:, :, :], in_=t_hs[:, :, :])
                nc.sync.dma_start(out=d_sxd.ap()[:, :, :], in_=t_sxs[:, :, :])
                nc.sync.dma_start(out=d_hpd.ap()[:, :, :, :],
                                  in_=t_hp[:, :, :, :])
                nc.sync.dma_start(out=d_d0d.ap()[:, :, :], in_=t_X[:, :, :])

    nc.compile()
    if os.environ.get("NN_ENC_NO_SWAP", "0") != "1":
        _sync_swap(nc)
    if os.environ.get("NN_ENC_ELIDE", "0") == "1":
        # NOTE: verified UNSAFE on real HW (NaN) — same-engine back-to-back
        # RAW needs the sem; kept only for experiments.
        _elide_same_engine_waits(nc)
    return nc


def _elide_same_engine_waits(nc):
    """Drop semaphore waits that are implied by same-engine program order.

    Engine queues execute in order and an op's memory write completes while
    the engine is busy (only the ack/sem visibility lags). So a wait on the
    engine's OWN counting semaphore, whose target count was already reached
    by increments from instructions earlier in this engine's stream, is
    redundant — remove it. Sems ever incremented by DMA-class instructions
    are excluded (DMA completion is asynchronous to queue order).
    """
    import concourse.mybir as mybir

    fn = nc.m.functions[0]
    dma_ops = {"DMACopy", "DMATrigger", "CollectiveCompute"}

    # pass 1: which sems are incremented by which engines / by DMAs
    sem_engines = {}
    sem_dma = set()
    for blk in fn.blocks:
        for i in blk.instructions:
            si = i.sync_info
            if si is None:
                continue
            for u in si.on_update:
                if u.sync_type != "semaphore":
                    continue
                sem_engines.setdefault(u.ant_name, set()).add(str(i.engine))
                if (i.opcode in dma_ops or "DMA" in i.opcode
                        or u.update_mode != "sem-inc"):
                    sem_dma.add(u.ant_name)

    allowed = set(os.environ.get("NN_ENC_ELIDE_ENG",
                                 "EngineType.DVE").split(","))
    nel = 0
    for blk in fn.blocks:
        counts = {}
        for i in blk.instructions:
            si = i.sync_info
            if si is None:
                continue
            waits = list(si.on_wait)
            if waits and str(i.engine) in allowed:
                eng = str(i.engine)
                keep = []
                for w in waits:
                    ok = (w.sync_type == "semaphore"
                          and w.wait_reg is None
                          and w.ant_name not in sem_dma
                          and sem_engines.get(w.ant_name) == {eng}
                          and w.wait_mode == "sem-ge-imm"
                          and counts.get(w.ant_name, 0) >= w.wait_value)
                    if ok:
                        nel += 1
                    else:
                        keep.append(w)
                if len(keep) != len(waits):
                    i.sync_info = mybir.SyncInfo(
                        on_wait=keep, on_update=list(si.on_update))
            for u in si.on_update:
                if u.sync_type == "semaphore" and u.update_mode == "sem-inc":
                    counts[u.ant_name] = counts.get(u.ant_name, 0) \
                        + u.update_value
    if DEBUG:
        print(f"_elide_same_engine_waits: {nel} waits elided")


def _sync_swap(nc):
    """Post-compile pass: when a spill EventSemaphore E carries the real
    (cross-engine) wait and its guarded instruction I carries only a trivial
    own-engine-counter wait, swap them. Both arrangements block I's execution
    until all waits are satisfied (E and I are adjacent on the same in-order
    queue), but with the chain wait attached to I, the sequencer can decode I
    while the wait is pending, hiding ~80ns of decode+dispatch latency per
    occurrence on the critical chain.
    """
    import concourse.mybir as mybir

    nswap = 0
    for blk in nc.m.functions[0].blocks:
        insts = list(blk.instructions)
        for n, e in enumerate(insts):
            if e.opcode != "EventSemaphore":
                continue
            se = e.sync_info
            if se is None or list(se.on_update):
                continue
            waits_e = list(se.on_wait)
            if not 1 <= len(waits_e) <= 2:
                continue
            if any(w.sync_type != "semaphore" or w.wait_reg is not None
                   for w in waits_e):
                continue
            eng = e.engine
            tgt = None
            for j in range(n + 1, min(n + 40, len(insts))):
                if insts[j].engine == eng:
                    if insts[j].opcode in ("EventSemaphore", "Ldweights"):
                        break
                    tgt = insts[j]
                    break
            if tgt is None:
                continue
            si = tgt.sync_info
            if si is None:
                continue
            waits_i = list(si.on_wait)
            if len(waits_i) != 1:
                continue
            w = waits_i[0]
            if w.sync_type != "semaphore" or w.wait_reg is not None:
                continue
            eng_name = str(eng).split(".")[-1]
            if not w.ant_name.startswith(eng_name):
                continue  # attached wait already the cross-engine chain wait
            if len(waits_e) == 1:
                mv, rest = waits_e[0], []
            else:
                # move the wait most likely to fire last onto I (prefer
                # non-PE: PE bank sems fire early in the step)
                ws = sorted(waits_e,
                            key=lambda x: 0 if x.ant_name.startswith("PE") else 1)
                rest, mv = [ws[0]], ws[1]
            e.sync_info = mybir.SyncInfo(on_wait=rest + [w], on_update=[])
            tgt.sync_info = mybir.SyncInfo(on_wait=[mv],
                                           on_update=list(si.on_update))
            nswap += 1

    # targeted pass: gate-chain instructions declare which producer's sem
    # must ride attached (resolved post-decode); all other waits move to the
    # spill EventSemaphore in front.
    want = dict(_CHAIN_FIX)
    nfix = 0
    for blk in nc.m.functions[0].blocks:
        insts = list(blk.instructions)
        pos = {i.name: n for n, i in enumerate(insts)}
        for name, pref in want.items():
            n = pos.get(name)
            if n is None:
                continue
            tgt = insts[n]
            si = tgt.sync_info
            if si is None:
                continue
            # nearest preceding same-engine EventSemaphore (the spill)
            spill = None
            for j in range(n - 1, max(-1, n - 30), -1):
                ij = insts[j]
                if ij.engine != tgt.engine:
                    continue
                if ij.opcode == "EventSemaphore":
                    sj = ij.sync_info
                    if sj is not None and not list(sj.on_update):
                        spill = ij
                break
            allw = list(si.on_wait) + (
                list(spill.sync_info.on_wait) if spill is not None else [])
            if any(w.sync_type != "semaphore" or w.wait_reg is not None
                   for w in allw):
                continue
            hit = [w for w in allw if w.ant_name.startswith(pref)]
            rest = [w for w in allw if not w.ant_name.startswith(pref)]
            if len(hit) != 1 or len(rest) > 2:
                continue
            if spill is None:
                continue  # single attached wait: nothing to rearrange
            tgt.sync_info = mybir.SyncInfo(on_wait=hit,
                                           on_update=list(si.on_update))
            spill.sync_info = mybir.SyncInfo(on_wait=rest, on_update=[])
            nfix += 1
    if DEBUG:
        print(f"_sync_swap: {nswap} swaps, {nfix} chain fixes")


def _prep_inputs(data, h0, gru_kernel, gru_rkernel, gru_bias,
                 w1_w, w1_b, w2_w, w2_b, v_w, v_b):
    f16 = np.float16
    f32 = np.float32

    import ml_dtypes
    R_all = np.ascontiguousarray(
        gru_rkernel.reshape(4, 128, 12, 128).transpose(1, 0, 2, 3))
    R8_l = R_all[:, :, 0:8, :].astype(ml_dtypes.float8_e4m3)
    Rh_l = (0.5 * R_all[:, :, 8:12, :]).astype(f16)
    K16 = (2.0 * gru_kernel).astype(f16)              # (256, 1536), doubled
    K_l = np.ascontiguousarray(
        K16.reshape(2, 128, 12, 128).transpose(1, 2, 0, 3))

    b_in, b_rec = gru_bias[0].astype(f32), gru_bias[1].astype(f32)

    s16 = np.zeros((128, S16_COLS), f16)
    s16[:, O_W1:O_W1 + 128] = w1_w.astype(f16)
    s16[:, O_W2:O_W2 + 512] = np.ascontiguousarray(
        w2_w.astype(f16).reshape(4, 128, 128).transpose(1, 0, 2)).reshape(128, 512)
    s16[:, O_ID:O_ID + 128] = np.eye(128, dtype=f16)
    s16[:, O_ONES:O_ONES + 128] = 1.0
    for j in range(4):
        for bb in range(4):
            s16[:, O_BRECH + 4 * j + bb] = b_rec[1024 + 128 * j:1024 + 128 * (j + 1)]

    s32 = np.zeros((128, S32_COLS), f32)
    s32[:, 0:8] = 2.0 * (b_in + b_rec)[:1024].reshape(8, 128).T
    s32[:, 8:12] = 2.0 * b_in[1024:].reshape(4, 128).T
    s32[:, 12] = w1_b.astype(f32)
    kap = np.asarray(SEP_KAP, np.float64)
    mu = np.asarray(SEP_MU, np.float64)
    s32[:, 13:26] = (kap[None, :] * w2_b.astype(np.float64)[:, None]
                     + mu[None, :]).astype(f32)
    s32[:, 26:41] = np.asarray(list(SEP_A) + [SEP_B0], f32)[None, :]
    s32[:, 41] = v_w[:, 0].astype(f32)

    tt = np.arange(T)
    per_core = []
    for c in range(NC):
        sl = slice(BL * c, BL * (c + 1))
        s16c = s16.copy()
        for j in range(4):
            for bb in range(BL):
                hcol = h0[BL * c + bb, 128 * j:128 * (j + 1)]
                s16c[:, O_H02 + 4 * j + bb] = 2.0 * hcol
                s16c[:, O_H016 + 4 * j + bb] = hcol
        # data16[t, l, n] = data[4c+l, t, n]
        data16 = np.ascontiguousarray(
            data[sl].astype(f16).transpose(1, 0, 2))
        # dataout[tt, l, n] = data[tt//4, 32*(tt%4) + 4c + l, n]
        dataout = np.empty((T, BL, N), f32)
        for l in range(BL):
            dataout[:, l, :] = data[tt // 4, 32 * (tt % 4) + 4 * c + l, :]
        per_core.append({
            "data16": data16, "dataout": dataout,
            "R8_l": R8_l, "Rh_l": Rh_l, "K_l": K_l,
            "small16": s16c, "small32": s32,
        })
    return per_core


def kernel(**inputs):
    from concourse.bass_utils import run_bass_kernel_spmd

    if "nc" not in _CACHE:
        _CACHE["nc"] = _build()
    nc = _CACHE["nc"]

    args = {k: np.asarray(v) for k, v in inputs.items()}
    per_core = _prep_inputs(
        args["data"], args["h0"], args["gru_kernel"], args["gru_rkernel"],
        args["gru_bias"], args["w1_w"], args["w1_b"], args["w2_w"],
        args["w2_b"], args["v_w"], args["v_b"])

    if "warm" not in _CACHE:
        # first execution after NEFF load can race; discard it
        run_bass_kernel_spmd(nc, per_core, core_ids=list(range(NC)))
        _CACHE["warm"] = True
    res = run_bass_kernel_spmd(nc, per_core, core_ids=list(range(NC)))
    _CACHE["last_res"] = res

    out = np.empty((B, T, N), np.float32)
    tt = np.arange(T)
    for c in range(NC):
        o = res.results[c]["out"]           # [BL, T(tt), N]
        for l in range(BL):
            out[tt // 4, 32 * (tt % 4) + 4 * c + l, :] = o[l]
    return out

